# revision 6
# baseline (speedup 1.0000x reference)
"""Trainium2 Bass kernel v2: full attention head, tensor-parallel over heads.

Per-core: 4 of 32 heads. Phases per core:
  A: qkv projection + rope (q/k transposed layout, v natural), bf16.
  B: causal attention; q/k/v loaded to SBUF once per batch; scores
     transposed s[k,q]; unnormalized p=exp(s/sqrt(d)); causal mask as
     0/1 multiply on p; denominator via DVE accumulation + one
     ones-column matmul per query group; normalization folded into yt
     eviction.
  C: output projection, stationary reused across 8 moving matmuls
     (tt,hj outer; oc inner over 8 PSUM banks).
Host sums the 8 bf16 partial outputs.
"""
import numpy as np

import concourse.bass as bass
import concourse.mybir as mybir
import concourse.tile as tile
from concourse import bacc, bass_utils
from concourse.bass import ts

F32 = mybir.dt.float32
F32R = mybir.dt.float32r
BF16 = mybir.dt.bfloat16

B = 2
S = 2048
EMB = 4096
NH = 32
HD = 128
N_CORES = 8
H_LOC = NH // N_CORES          # 4 heads per core
FLOC = H_LOC * HD              # 512
INV_SQRT_HD = 1.0 / float(np.sqrt(HD))

TCH = 1024                     # phase-A token chunk
NE = EMB // 128                # 32 e-tiles
NXP = 4                        # x parts per chunk
EPP = NE // NXP                # 8 e-tiles per x part
NKT = S // 128                 # 16 key tiles
NTT = S // 128                 # 16 token tiles
NQG = S // 512                 # 4 query groups

SHUF_MASK = list(range(16, 32)) + list(range(0, 16))


def _rope_perm():
    perm = np.zeros(HD, dtype=np.int64)
    for q in range(4):
        for r in range(16):
            perm[32 * q + r] = 2 * (16 * q + r)
            perm[32 * q + 16 + r] = 2 * (16 * q + r) + 1
    return perm


def host_prep(x, w_atten, w_proj, freqs_cos, freqs_sin):
    import ml_dtypes
    BF = ml_dtypes.bfloat16
    perm = _rope_perm()

    # x chunks: [2B, 128, NE*TCH] contiguous per chunk
    xt = x.transpose(0, 2, 1).reshape(B, NE, 128, S).transpose(0, 2, 1, 3)
    # [B,128,NE,S] -> chunk ch=(b,ci): [:, :, ci*TCH:...] -> [128, NE*TCH]
    xp = np.empty((2 * B, 128, NE * TCH), dtype=BF)
    for b in range(B):
        for ci in range(S // TCH):
            xp[2 * b + ci] = (
                xt[b, :, :, ci * TCH:(ci + 1) * TCH].reshape(128, NE * TCH))

    cs = np.zeros((HD, S), dtype=np.float32)
    ss = np.zeros((HD, S), dtype=np.float32)
    cosT = freqs_cos.T
    sinT = freqs_sin.T
    for q in range(4):
        for r in range(16):
            i = 16 * q + r
            cs[32 * q + r] = cosT[i]
            cs[32 * q + 16 + r] = cosT[i]
            ss[32 * q + r] = -sinT[i]
            ss[32 * q + 16 + r] = sinT[i]

    # 0/1 causal masks for the 4 diagonal 128x512 tiles
    m01 = np.zeros((4, 128, 512), dtype=np.float32)
    p_idx = np.arange(128)[:, None]
    q_idx = np.arange(512)[None, :]
    for m in range(4):
        m01[m] = (q_idx >= p_idx + 128 * m).astype(np.float32)

    shared = {
        "xp": xp,
        "cs": np.ascontiguousarray(cs).astype(BF),
        "ss": np.ascontiguousarray(ss).astype(BF),
        "m01": m01.astype(BF),
        "ones_col": np.ones((128, 1), dtype=np.float32).astype(BF),
        "ones_row": np.ones((1, 128), dtype=np.float32),
    }

    per_core = []
    for c in range(N_CORES):
        h0 = c * H_LOC
        # wq/wk: per-fi packed [H_LOC, 128, NE*128], rope-permuted cols
        wq = np.empty((H_LOC, 128, NE * 128), dtype=BF)
        wk = np.empty((H_LOC, 128, NE * 128), dtype=BF)
        for j in range(H_LOC):
            qcols = (h0 + j) * HD + perm
            wqj = w_atten[:, qcols]              # [EMB, 128]
            wkj = w_atten[:, EMB + qcols]
            wq[j] = wqj.reshape(NE, 128, 128).transpose(1, 0, 2).reshape(
                128, NE * 128)
            wk[j] = wkj.reshape(NE, 128, 128).transpose(1, 0, 2).reshape(
                128, NE * 128)
        wv_n = w_atten[:, 2 * EMB + h0 * HD: 2 * EMB + (h0 + H_LOC) * HD]
        wv = np.ascontiguousarray(
            wv_n.reshape(NE, 128, FLOC).transpose(1, 0, 2).reshape(
                128, NE * FLOC)).astype(BF)
        # wp: [H_LOC, 128, EMB]
        wp = np.ascontiguousarray(
            w_proj[h0 * HD:(h0 + H_LOC) * HD, :].reshape(
                H_LOC, 128, EMB)).astype(BF)
        per_core.append({"wq": wq, "wk": wk, "wv": wv, "wp": wp})
    return shared, per_core


def build_nc(reps=1):
    nc = bacc.Bacc("TRN2", target_bir_lowering=False, debug=False)

    xp = nc.dram_tensor("xp", [2 * B, 128, NE * TCH], BF16,
                        kind="ExternalInput")
    wq = nc.dram_tensor("wq", [H_LOC, 128, NE * 128], BF16,
                        kind="ExternalInput")
    wk = nc.dram_tensor("wk", [H_LOC, 128, NE * 128], BF16,
                        kind="ExternalInput")
    wv = nc.dram_tensor("wv", [128, NE * FLOC], BF16, kind="ExternalInput")
    wp = nc.dram_tensor("wp", [H_LOC, 128, EMB], BF16, kind="ExternalInput")
    cs = nc.dram_tensor("cs", [128, S], BF16, kind="ExternalInput")
    ss_t = nc.dram_tensor("ss", [128, S], BF16, kind="ExternalInput")
    m01 = nc.dram_tensor("m01", [4, 128, 512], BF16, kind="ExternalInput")
    ones_col = nc.dram_tensor("ones_col", [128, 1], BF16,
                              kind="ExternalInput")
    ones_row = nc.dram_tensor("ones_row", [1, 128], F32R,
                              kind="ExternalInput")
    out = nc.dram_tensor("out", [B, S, EMB], BF16, kind="ExternalOutput")

    qt_d = [nc.dram_tensor(f"qt_d{b}", [FLOC, S], BF16, kind="Internal")
            for b in range(B)]
    kt_d = [nc.dram_tensor(f"kt_d{b}", [FLOC, S], BF16, kind="Internal")
            for b in range(B)]
    v_d = [nc.dram_tensor(f"v_d{b}", [NKT, 128, FLOC], BF16, kind="Internal")
           for b in range(B)]

    with tile.TileContext(nc) as tc, \
         nc.allow_low_precision(reason="bf16 storage within error budget; "
                                "all matmul accumulation stays f32 in PSUM"):
      for rep in range(reps):
        with tc.tile_pool(name=f"pers{rep}", bufs=1) as pers:
            oc_sb = pers.tile([128, 1], BF16, tag="ones_col")
            or_sb = pers.tile([1, 128], F32R, tag="ones_row")
            cs_sb = pers.tile([128, S], BF16, tag="cs")
            ss_sb = pers.tile([128, S], BF16, tag="ss")
            m01_sb = pers.tile([128, 4 * 512], BF16, tag="m01")
            nc.scalar.dma_start(oc_sb[:], ones_col.ap()[:])
            nc.scalar.dma_start(or_sb[:], ones_row.ap()[:])
            nc.scalar.dma_start(cs_sb[:], cs.ap()[:])
            nc.scalar.dma_start(ss_sb[:], ss_t.ap()[:])
            for m in range(4):
                nc.scalar.dma_start(m01_sb[:, ts(m, 512)], m01.ap()[m])

            # ================= PHASE A: qkv projection + rope =============
            with tc.tile_pool(name=f"pa_x{rep}", bufs=2) as pax, \
                 tc.tile_pool(name=f"pa_w{rep}", bufs=2) as paw, \
                 tc.tile_pool(name=f"pa_wv{rep}", bufs=1) as pawv, \
                 tc.tile_pool(name=f"pa_t{rep}", bufs=3) as pat, \
                 tc.tile_pool(name=f"pa_ps{rep}", bufs=2, space="PSUM") as paps, \
                 tc.tile_pool(name=f"pa_psv{rep}", bufs=1, space="PSUM") as papsv:
                for ch in range(2 * B):
                    b, s0 = ch // 2, (ch % 2) * TCH
                    x_parts = []

                    def load_x(xi, split=False):
                        xp_t = pax.tile([128, EPP * TCH], BF16, tag=f"x{xi}",
                                        name=f"x{xi}_{ch}_{rep}")
                        base = xi * EPP * TCH
                        if split:
                            hlen = EPP * TCH // 2
                            nc.sync.dma_start(
                                xp_t[:, 0:hlen],
                                xp.ap()[ch, :, base:base + hlen])
                            nc.sync.dma_start(
                                xp_t[:, hlen:2 * hlen],
                                xp.ap()[ch, :, base + hlen:base + 2 * hlen])
                        else:
                            nc.sync.dma_start(
                                xp_t[:],
                                xp.ap()[ch, :, base:base + EPP * TCH])
                        x_parts.append(xp_t)

                    wt0 = paw.tile([128, NE * 128], BF16, tag="wqk",
                                   name=f"wqk_{ch}_0_{rep}")
                    nc.sync.dma_start(wt0[:], wq.ap()[0])
                    load_x(0, split=(ch == 0))

                    def x_slice(e, lo, ln):
                        t_ = x_parts[e // EPP]
                        base = (e % EPP) * TCH
                        return t_[:, base + lo: base + lo + ln]

                    # ---- q/k (transposed layout + rope) ----
                    for fi in range(2 * H_LOC):
                        if fi == 0:
                            wt = wt0
                        else:
                            wt = paw.tile([128, NE * 128], BF16, tag="wqk",
                                          name=f"wqk_{ch}_{fi}_{rep}")
                            nc.sync.dma_start(
                                wt[:],
                                (wq if fi < H_LOC else wk).ap()[fi % H_LOC])
                        if fi == 0:
                            load_x(1)
                            load_x(2)
                            load_x(3)
                        if fi == H_LOC:
                            wv_t = pawv.tile([128, NE * FLOC], BF16, tag="wv",
                                             name=f"wv_{ch}_{rep}")
                            nc.sync.dma_start(wv_t[:], wv.ap()[:])
                        f0 = (fi % H_LOC) * 128
                        ps = paps.tile([128, TCH], F32, tag="qk_ps",
                                       name=f"qk_ps_{ch}_{fi}_{rep}")
                        for e in range(NE):
                            for hh in range(TCH // 512):
                                nc.tensor.matmul(
                                    ps[:, ts(hh, 512)],
                                    wt[:, ts(e, 128)],
                                    x_slice(e, hh * 512, 512),
                                    start=(e == 0), stop=(e == NE - 1))
                        dst = qt_d[b] if fi < H_LOC else kt_d[b]
                        for hh in range(TCH // 512):
                            raw = pat.tile([128, 512], F32, tag="raw")
                            nc.scalar.copy(raw[:], ps[:, ts(hh, 512)])
                            shuf = pat.tile([128, 512], F32, tag="shuf")
                            nc.vector.stream_shuffle(shuf[:], raw[:], SHUF_MASK)
                            nc.vector.tensor_mul(
                                raw[:], raw[:], cs_sb[:, s0 + hh * 512:
                                                      s0 + (hh + 1) * 512])
                            nc.vector.tensor_mul(
                                shuf[:], shuf[:], ss_sb[:, s0 + hh * 512:
                                                        s0 + (hh + 1) * 512])
                            rope = pat.tile([128, 512], BF16, tag="rope")
                            nc.vector.tensor_add(rope[:], raw[:], shuf[:])
                            nc.scalar.dma_start(
                                dst.ap()[f0:f0 + 128,
                                         s0 + hh * 512: s0 + (hh + 1) * 512],
                                rope[:])
                    # ---- v (natural layout), 2 half-passes of 4 token tiles
                    for half in range(2):
                        ps_v = []
                        for tt in range(4):
                            ps_v.append(papsv.tile(
                                [128, FLOC], F32, tag=f"v_ps{tt}",
                                name=f"v_ps{tt}_{ch}_{half}_{rep}"))
                        for e in range(NE):
                            for tt in range(4):
                                nc.tensor.matmul(
                                    ps_v[tt][:],
                                    x_slice(e, half * 512 + tt * 128, 128),
                                    wv_t[:, ts(e, FLOC)],
                                    start=(e == 0), stop=(e == NE - 1))
                        for tt in range(4):
                            v_out = pat.tile([128, FLOC], BF16, tag="v_out")
                            nc.scalar.copy(v_out[:], ps_v[tt][:])
                            jt = (s0 + half * 512 + tt * 128) // 128
                            nc.scalar.dma_start(v_d[b].ap()[jt], v_out[:])

            # ================= PHASE B: causal attention ==================
            # yt stays in SBUF across B->C (no DRAM round-trip)
            yt_tiles = {}
            pyt_ctx = tc.tile_pool(name=f"pb_yt{rep}", bufs=2)
            pyt = pyt_ctx.__enter__()
            with tc.tile_pool(name=f"pb_qk{rep}", bufs=2) as pbqk, \
                 tc.tile_pool(name=f"pb_p{rep}", bufs=5) as pbp, \
                 tc.tile_pool(name=f"pb_s{rep}", bufs=3, space="PSUM") as pbs, \
                 tc.tile_pool(name=f"pb_y{rep}", bufs=2, space="PSUM") as pby, \
                 tc.tile_pool(name=f"pb_d{rep}", bufs=2, space="PSUM") as pbd, \
                 tc.tile_pool(name=f"pb_b{rep}", bufs=1, space="PSUM") as pbb:
              for b in range(B):
                    q_sb, k_sb = [], []
                    for h in range(H_LOC):
                        qh = pbqk.tile([128, S], BF16, tag=f"q{h}",
                                       name=f"q{h}_{b}_{rep}")
                        kh = pbqk.tile([128, S], BF16, tag=f"k{h}",
                                       name=f"k{h}_{b}_{rep}")
                        nc.gpsimd.dma_start(qh[:],
                                            qt_d[b].ap()[h * 128:(h + 1) * 128, :])
                        nc.gpsimd.dma_start(kh[:],
                                            kt_d[b].ap()[h * 128:(h + 1) * 128, :])
                        q_sb.append(qh)
                        k_sb.append(kh)
                    v_sb = pbqk.tile([128, NKT * FLOC], BF16, tag="v",
                                     name=f"v_{b}_{rep}")
                    for jt in range(NKT):
                        nc.gpsimd.dma_start(v_sb[:, ts(jt, FLOC)],
                                            v_d[b].ap()[jt])

                    for h in range(H_LOC):
                        yt_h = pyt.tile([128, S], BF16, tag=f"yt{h}",
                                        name=f"ytsb_{b}_{h}_{rep}")
                        yt_tiles[(b, h)] = yt_h
                        for pair in ((0, 1), (2, 3)):
                            njmax = 4 * pair[1] + 4
                            y_ps = {}
                            den = {}
                            for g in pair:
                                y_ps[g] = pby.tile(
                                    [128, 512], F32, tag="y",
                                    name=f"y_{b}_{h}_{g}_{rep}")
                                den[g] = pbd.tile(
                                    [1, 512], F32, tag="den",
                                    name=f"den_{b}_{h}_{g}_{rep}")
                            prev = None  # (j, {g: p_tile})

                            def flush(prev_j, pmap):
                                for g, p_t in pmap.items():
                                    nc.tensor.matmul(
                                        y_ps[g][:],
                                        v_sb[:, prev_j * FLOC + h * 128:
                                             prev_j * FLOC + h * 128 + 128],
                                        p_t[:],
                                        start=(prev_j == 0),
                                        stop=(prev_j == 4 * g + 3))
                                for g, p_t in pmap.items():
                                    nc.tensor.matmul(
                                        den[g][:], oc_sb[:], p_t[:],
                                        start=(prev_j == 0),
                                        stop=(prev_j == 4 * g + 3))

                            def tail(g):
                                recip = pbp.tile([1, 512], F32R, tag="recip")
                                nc.vector.reciprocal(recip[:], den[g][:])
                                bc_ps = pbb.tile([128, 512], F32, tag="bc",
                                                 name=f"bc_{b}_{h}_{g}_{rep}")
                                nc.tensor.matmul(bc_ps[:], or_sb[:], recip[:],
                                                 start=True, stop=True)
                                bc_sb = pbp.tile([128, 512], F32R, tag="bc_sb")
                                nc.vector.tensor_copy(bc_sb[:], bc_ps[:])
                                nc.vector.tensor_mul(
                                    yt_h[:, g * 512:(g + 1) * 512],
                                    y_ps[g][:], bc_sb[:])
                            # (yt stays in SBUF; no store)

                            for j in range(njmax):
                                gs = [g for g in pair if j < 4 * g + 4]
                                pmap = {}
                                for g in gs:
                                    s_ps = pbs.tile(
                                        [128, 512], F32, tag="s",
                                        name=f"s_{b}_{h}_{g}_{j}_{rep}")
                                    nc.tensor.matmul(
                                        s_ps[:], k_sb[h][:, ts(j, 128)],
                                        q_sb[h][:, ts(g, 512)],
                                        start=True, stop=True)
                                    p_t = pbp.tile([128, 512], BF16, tag="p")
                                    nc.scalar.activation(
                                        p_t[:], s_ps[:],
                                        mybir.ActivationFunctionType.Exp,
                                        scale=INV_SQRT_HD)
                                    m = j - 4 * g
                                    if m >= 0:
                                        # only cols < 128*(m+1) can be masked
                                        w_m = 128 * (m + 1)
                                        nc.vector.tensor_mul(
                                            p_t[:, 0:w_m], p_t[:, 0:w_m],
                                            m01_sb[:, m * 512:m * 512 + w_m])
                                    pmap[g] = p_t
                                if prev is not None:
                                    flush(*prev)
                                    if prev[0] == 4 * pair[0] + 3:
                                        tail(pair[0])
                                prev = (j, pmap)
                            flush(*prev)
                            tail(pair[1])

            # ================= PHASE C: output projection =================
            with tc.tile_pool(name=f"pc_w{rep}", bufs=1) as pcw, \
                 tc.tile_pool(name=f"pc_o{rep}", bufs=4) as pco, \
                 tc.tile_pool(name=f"pc_ps{rep}", bufs=2, space="PSUM") as pcps:
                wp_sb = []
                for hj in range(H_LOC):
                    w_t = pcw.tile([128, EMB], BF16, tag=f"wp{hj}",
                                   name=f"wp{hj}_{rep}")
                    nc.gpsimd.dma_start(w_t[:], wp.ap()[hj])
                    wp_sb.append(w_t)
                for b in range(B):
                    yt_sb = [yt_tiles[(b, hj)] for hj in range(H_LOC)]
                    for half in range(2):
                        for tt in range(NTT):
                            ps_oc = []
                            for oc4 in range(4):
                                ps_oc.append(pcps.tile(
                                    [128, 512], F32, tag=f"o{oc4}",
                                    name=f"o{oc4}_{b}_{half}_{tt}_{rep}"))
                            for hj in range(H_LOC):
                                for oc4 in range(4):
                                    oc2 = half * 4 + oc4
                                    nc.tensor.matmul(
                                        ps_oc[oc4][:],
                                        yt_sb[hj][:, ts(tt, 128)],
                                        wp_sb[hj][:, ts(oc2, 512)],
                                        start=(hj == 0), stop=(hj == H_LOC - 1))
                            for oc4 in range(4):
                                oc2 = half * 4 + oc4
                                o_t = pco.tile([128, 512], BF16, tag="o_t")
                                if oc4 % 2 == 0:
                                    nc.scalar.copy(o_t[:], ps_oc[oc4][:])
                                else:
                                    nc.vector.tensor_copy(o_t[:], ps_oc[oc4][:])
                                nc.sync.dma_start(
                                    out.ap()[b, tt * 128:(tt + 1) * 128,
                                             oc2 * 512:(oc2 + 1) * 512],
                                    o_t[:])
            pyt_ctx.__exit__(None, None, None)

    nc.compile()
    return nc


_NC_CACHE = None


def kernel(x, w_atten, w_proj, freqs_cos, freqs_sin):
    global _NC_CACHE
    x = np.asarray(x, dtype=np.float32)
    w_atten = np.asarray(w_atten, dtype=np.float32)
    w_proj = np.asarray(w_proj, dtype=np.float32)
    freqs_cos = np.asarray(freqs_cos, dtype=np.float32)
    freqs_sin = np.asarray(freqs_sin, dtype=np.float32)

    shared, per_core = host_prep(x, w_atten, w_proj, freqs_cos, freqs_sin)
    if _NC_CACHE is None:
        _NC_CACHE = build_nc()
    nc = _NC_CACHE
    in_maps = [{**shared, **per_core[c]} for c in range(N_CORES)]
    res = bass_utils.run_bass_kernel_spmd(nc, in_maps,
                                          core_ids=list(range(N_CORES)))
    acc = np.zeros((B, S, EMB), dtype=np.float64)
    for c in range(N_CORES):
        acc += res.results[c]["out"].astype(np.float64)
    return acc.astype(np.float32)
